# revision 31
# baseline (speedup 1.0000x reference)
"""Grouped BERT self-attention on 8 TRN2 NeuronCores.

Problem: G=4 groups, B=4 batch, L=512 seq, C=768 (12 heads x 64).
Sharding: the 16 (g, b) attention problems are embarrassingly parallel;
each core handles one group g = core//2 and two batches. Weights are
per-group so each core loads exactly one group's weights. No collectives.

Per-(g,b) on-chip dataflow (bf16 matmul inputs, fp32 accumulation):
  qT[d,l] = Wq[c,d].T @ hsT[c,l]    (weights in natural layout = lhsT;
                                     bias folded into PSUM->SBUF copy)
  kT[d,l] = Wk[c,d].T @ hsT[c,l]
  v[m,d]  = hsT[c,m].T @ Wv[c,d]    (+bias, stored [m, head, 65] with a
                                     ones column per head -> softmax denom)
  ST[m,l] = kT[d,m].T @ qT[d,l]     (heads paired on partitions 0:64/64:128
                                     -> concurrent PE row-tiles, shared
                                     2-bank PSUM tile)
  E[m,l]  = exp(0.125*ST + mask[m]) (one ACT op per head-pair, bf16 out)
  ctx[l, 2, d+1] = E[m,l].T @ v_aug[m, d+1]  (ctx-direct: E chunk is the
                                     stationary operand, head pair shares
                                     one PSUM bank; column d=64 catches
                                     the softmax denominator)
  out[l,:] = ctx * recip(denom)     (one [128,2] reciprocal + one
                                     broadcast-multiply per pair/l-chunk)

PE emission interleaves score-pair matmuls of unit N with the PV matmuls
of unit N-2 so the in-order PE queue never waits on the ScalarEngine's
exp. Input DMAs are split/staged so the tensors gating the first matmul
group get full HBM bandwidth, and dummy warm-up matmuls hold the PE
activity monitor at full clock until real work arrives.
"""

import numpy as np
import ml_dtypes

import concourse.bacc as bacc
import concourse.bass as bass
import concourse.tile as tile
import concourse.mybir as mybir
from concourse import bass_utils

# avoid FishPath artifact upload in the axon trace path
bass_utils.upload_artifacts = lambda tmpdir: tmpdir

G, B, L, C = 4, 4, 512, 768
NH, DH = 12, 64
NB = 2          # batches per core
CCH = C // 128  # 6 contraction chunks
LCH = L // 128  # 4 seq chunks
N_CORES = 8

BF16 = mybir.dt.bfloat16
F32 = mybir.dt.float32
NPBF16 = ml_dtypes.bfloat16

_COMPILED = None


def _build():
    nc = bacc.Bacc("TRN2", target_bir_lowering=False, debug=False)
    AF = mybir.ActivationFunctionType

    hst_d = nc.declare_dram_parameter("hst", [NB, 128, CCH, L], BF16, isOutput=False)
    wq_d = nc.declare_dram_parameter("wq", [CCH, 128, CCH, 128], BF16, isOutput=False)
    wk_d = nc.declare_dram_parameter("wk", [CCH, 128, CCH, 128], BF16, isOutput=False)
    wv_d = nc.declare_dram_parameter("wv", [128, CCH, C], BF16, isOutput=False)
    bq_d = nc.declare_dram_parameter("bq", [128, CCH], F32, isOutput=False)
    bk_d = nc.declare_dram_parameter("bk", [128, CCH], F32, isOutput=False)
    bvb_d = nc.declare_dram_parameter("bvb", [128, C], BF16, isOutput=False)
    mask_d = nc.declare_dram_parameter("mask", [NB, 128, LCH], F32, isOutput=False)
    out_d = nc.declare_dram_parameter("out", [NB, LCH, 128, C], BF16, isOutput=True)

    with tile.TileContext(nc) as tc:
        with (
            tc.tile_pool(name="wpool", bufs=1) as wpool,
            tc.tile_pool(name="hpool", bufs=2) as hpool,
            tc.tile_pool(name="qkpool", bufs=2) as qkpool,
            tc.tile_pool(name="vpool", bufs=2 * LCH) as vpool,
            tc.tile_pool(name="epool", bufs=12) as epool,
            tc.tile_pool(name="cpool", bufs=2 * LCH) as cpool,
            tc.tile_pool(name="rpool", bufs=8) as rpool,
            tc.tile_pool(name="pqk", bufs=2, space=bass.MemorySpace.PSUM) as pqk,
            tc.tile_pool(name="pss", bufs=2, space=bass.MemorySpace.PSUM) as pss_pool,
            tc.tile_pool(name="ppv", bufs=2, space=bass.MemorySpace.PSUM) as ppv,
        ):
            # ---- persistent constants ----
            wq = wpool.tile([128, CCH, CCH, 128], BF16, tag="wq")
            wk = wpool.tile([128, CCH, CCH, 128], BF16, tag="wk")
            wv = wpool.tile([128, CCH, C], BF16, tag="wv")
            bq = wpool.tile([128, CCH], F32, tag="bq")
            bk = wpool.tile([128, CCH], F32, tag="bk")
            bvb = wpool.tile([128, C], BF16, tag="bvb")
            # one big DMA per tensor (DMA *issue* costs ~0.7us of queue
            # time each); weights on the sync queue, activations on the
            # scalar queue so the first matmul's inputs land in parallel.
            hst, msk, qt, kt, vt, e_of = {}, {}, {}, {}, {}, {}
            for b in range(NB):
                hst[b] = hpool.tile([128, CCH, L], BF16, tag="hst", name=f"hst{b}")
                msk[b] = hpool.tile([128, LCH], F32, tag="mask", name=f"msk{b}")

            # staged input DMAs: wq + hst0 gate the first matmul group, so
            # they transfer first at full HBM bandwidth; wk, wv, hst1 are
            # chained behind earlier transfers via explicit dependency
            # edges. Small tensors ride the otherwise-idle GPSIMD queue.
            h = CCH // 2
            d_wq0 = nc.sync.dma_start(wq[:, 0], wq_d[0])
            d_wk0 = nc.sync.dma_start(wk[:, 0], wk_d[0])
            d_h0 = [nc.scalar.dma_start(hst[0][:, 2 * i : 2 * i + 2], hst_d[0, :, 2 * i : 2 * i + 2])
                    for i in range(3)]
            nc.gpsimd.dma_start(msk[0][:], mask_d[0])
            nc.gpsimd.dma_start(bq[:], bq_d[:])
            nc.gpsimd.dma_start(bk[:], bk_d[:])
            nc.gpsimd.dma_start(bvb[:], bvb_d[:])
            nc.gpsimd.dma_start(msk[1][:], mask_d[1])
            d_wqA = nc.sync.dma_start(wq[:, 1:3], wq_d[1:3].rearrange("j p k d -> p j k d"))
            d_wkA = nc.sync.dma_start(wk[:, 1:3], wk_d[1:3].rearrange("j p k d -> p j k d"))
            d_wqB = nc.sync.dma_start(wq[:, 3:6], wq_d[3:6].rearrange("j p k d -> p j k d"))
            d_wkB = nc.sync.dma_start(wk[:, 3:6], wk_d[3:6].rearrange("j p k d -> p j k d"))
            d_wv = [nc.sync.dma_start(wv[:, 2 * i : 2 * i + 2], wv_d[:, 2 * i : 2 * i + 2])
                    for i in range(3)]
            d_h1 = [nc.scalar.dma_start(hst[1][:, 0:h], hst_d[1, :, 0:h]),
                    nc.scalar.dma_start(hst[1][:, h:], hst_d[1, :, h:])]
            for i in range(3):
                bass._add_dep_helper(d_wv[i].ins, d_h0[i].ins, True, "wv after hst0")
            for i in range(2):
                bass._add_dep_helper(d_h1[i].ins, d_h0[i].ins, True, "hst1 after hst0")

            def emit_v(b):
                vt[b] = [
                    vpool.tile([128, NH, DH + 1], BF16, tag="v", name=f"v{b}_{t}")
                    for t in range(LCH)
                ]
                for t in range(LCH):
                    for half in range(2):
                        ncol = C // 2  # 384
                        ps = pqk.tile([128, ncol], F32, tag="big", name="psv")
                        for k in range(CCH):
                            nc.tensor.matmul(
                                ps[:],
                                hst[b][:, k, 128 * t : 128 * (t + 1)],
                                wv[:, k, half * ncol : (half + 1) * ncol],
                                start=(k == 0),
                                stop=(k == CCH - 1),
                            )
                        nh2 = NH // 2
                        nc.vector.tensor_add(
                            vt[b][t][:, half * nh2 : (half + 1) * nh2, 0:DH],
                            ps[:].rearrange("p (h d) -> p h d", d=DH),
                            bvb[:, half * ncol : (half + 1) * ncol].rearrange(
                                "p (h d) -> p h d", d=DH
                            ),
                        )
                    nc.vector.memset(vt[b][t][:, :, DH : DH + 1], 1.0)

            def emit_qk_chunk(b, j):
                if j == 0:
                    qt[b] = qkpool.tile([128, CCH, L], BF16, tag="qt", name=f"qt{b}")
                    kt[b] = qkpool.tile([128, CCH, L], BF16, tag="kt", name=f"kt{b}")
                for w, bias, dst in ((wq, bq, qt[b]), (wk, bk, kt[b])):
                    ps = pqk.tile([128, L], F32, tag="big", name="psqk")
                    for k in range(CCH):
                        nc.tensor.matmul(
                            ps[:],
                            w[:, j, k, :],
                            hst[b][:, k, :],
                            start=(k == 0),
                            stop=(k == CCH - 1),
                        )
                    nc.vector.tensor_scalar_add(dst[:, j, :], ps[:], bias[:, j : j + 1])

            def emit_scores_mc(b, hp, mc):
                if mc == 0:
                    e_of[(b, hp)] = [
                        epool.tile([128, 2, L], BF16, tag="e", name=f"e{b}_{hp}_{m}")
                        for m in range(LCH)
                    ]
                e = e_of[(b, hp)]
                ps = pss_pool.tile([128, 2, L], F32, tag="pss", name="pss")
                for h2 in range(2):
                    pr = slice(64 * h2, 64 * (h2 + 1))
                    nc.tensor.matmul(
                        ps[:, h2, :],
                        kt[b][pr, hp, 128 * mc : 128 * (mc + 1)],
                        qt[b][pr, hp, :],
                    )
                nc.scalar.activation(
                    e[mc][:], ps[:], AF.Exp,
                    bias=msk[b][:, mc : mc + 1], scale=0.125,
                )

            def emit_pv_pair(b, hp, lcs):
                # ctx-direct PV for a head pair over l-chunks lcs: one PSUM
                # bank holds both heads' [128, 65] results; denominators at
                # column 64 of each -> one [128,2] reciprocal + one
                # broadcast-multiply into the assembled ctx tile.
                e = e_of[(b, hp)]
                for lc in lcs:
                    pc = ppv.tile([128, 2, DH + 1], F32, tag="pv", name="pc")
                    for h2 in range(2):
                        head = 2 * hp + h2
                        for mc in range(LCH):
                            nc.tensor.matmul(
                                pc[:, h2, :],
                                e[mc][:, h2, 128 * lc : 128 * (lc + 1)],
                                vt[b][mc][:, head, :],
                                start=(mc == 0),
                                stop=(mc == LCH - 1),
                            )
                    rec = rpool.tile([128, 2, 1], F32, tag="rec", name="rec")
                    nc.vector.reciprocal(rec[:], pc[:, :, DH : DH + 1])
                    nc.vector.tensor_mul(
                        ctxs[(b, lc)][:, 2 * hp : 2 * hp + 2, :],
                        pc[:, :, 0:DH],
                        rec[:].broadcast_to((128, 2, DH)),
                    )

            # ---- HAM warm-up: dummy matmuls on garbage SBUF keep the PE
            # active (half-clock) from program start so the activity monitor
            # releases the clock gate before the first real matmul ----
            warm = wpool.tile([128, L], BF16, tag="warm")
            nc.vector.memset(warm[:], 0.0)
            for i in range(24):
                pw = ppv.tile([1, L], F32, tag="pv", name=f"pw{i}")
                nc.tensor.matmul(pw[:], warm[:, 0:1], warm[:])

            # ---- emission schedule ----
            units = []
            for b in range(NB):
                for hp in range(CCH):
                    units.append((b, hp))

            def emit_out(b):
                for lc in range(LCH):
                    flat = ctxs[(b, lc)][:].rearrange("p h d -> p (h d)")
                    nc.sync.dma_start(out_d[b, lc, :, 0 : C // 2], flat[:, 0 : C // 2])
                    nc.scalar.dma_start(out_d[b, lc, :, C // 2 :], flat[:, C // 2 :])

            ctxs = {}
            for b in range(NB):
                for lc in range(LCH):
                    ctxs[(b, lc)] = cpool.tile(
                        [128, NH, DH], BF16, tag="ctx", name=f"ctx{b}_{lc}"
                    )
            from collections import deque
            # hand-scheduled prologue: no PV work exists yet to cover the
            # ScalarEngine's exp latency, so interleave qk(0,1) and v(0)
            # between score groups as PE filler
            emit_qk_chunk(0, 0)
            emit_scores_mc(0, 0, 0)
            emit_scores_mc(0, 0, 1)
            emit_qk_chunk(0, 1)
            emit_scores_mc(0, 0, 2)
            emit_scores_mc(0, 0, 3)
            emit_scores_mc(0, 1, 0)
            emit_scores_mc(0, 1, 1)
            emit_v(0)
            emit_scores_mc(0, 1, 2)
            emit_scores_mc(0, 1, 3)
            lag = deque([(0, 0), (0, 1)])
            for b, hp in units[2:]:
                if b == 1 and hp == 0:
                    emit_v(1)
                emit_qk_chunk(b, hp)
                emit_scores_mc(b, hp, 0)
                emit_scores_mc(b, hp, 1)
                if len(lag) >= 2:
                    emit_pv_pair(*lag[0], (0, 1))
                emit_scores_mc(b, hp, 2)
                emit_scores_mc(b, hp, 3)
                if len(lag) >= 2:
                    pp = lag.popleft()
                    emit_pv_pair(*pp, (2, 3))
                    e_of.pop(pp)
                    if pp[1] == CCH - 1:
                        emit_out(pp[0])
                lag.append((b, hp))
            while lag:
                pp = lag.popleft()
                emit_pv_pair(*pp, (0, 1))
                emit_pv_pair(*pp, (2, 3))
                e_of.pop(pp)
                if pp[1] == CCH - 1:
                    emit_out(pp[0])

    nc.compile()
    return nc


def _get_compiled():
    global _COMPILED
    if _COMPILED is None:
        _COMPILED = _build()
    return _COMPILED


def _prep_core(hs, mask, wq, wk, wv, bq, bk, bv, g, b0):
    hs_gb = np.ascontiguousarray(hs[g, b0 : b0 + NB])  # [2, L, C]
    # hst[b, p, j, l] = hs[g, b0+b, l, 128j+p]
    hst = np.ascontiguousarray(
        hs_gb.transpose(0, 2, 1).reshape(NB, CCH, 128, L).transpose(0, 2, 1, 3)
    ).astype(NPBF16)

    def wprep(w):
        # k-chunk-major: [p, k, d] = W[128k+p, d]
        return np.ascontiguousarray(
            w[g].reshape(CCH, 128, C).transpose(1, 0, 2)
        ).astype(NPBF16)

    def wprep_d(w):
        # d-chunk-major: [j, p, k, d'] = W[128k+p, 128j+d']
        return np.ascontiguousarray(
            w[g].reshape(CCH, 128, CCH, 128).transpose(2, 1, 0, 3)
        ).astype(NPBF16)

    bq_t = np.ascontiguousarray(bq[g, 0].reshape(CCH, 128).T).astype(np.float32)
    bk_t = np.ascontiguousarray(bk[g, 0].reshape(CCH, 128).T).astype(np.float32)
    bvb = np.ascontiguousarray(np.broadcast_to(bv[g, 0], (128, C))).astype(NPBF16)
    # mask[b, p, mc] = mask[g, b0+b, 0, 0, 128mc+p]
    msk = np.ascontiguousarray(
        mask[g, b0 : b0 + NB, 0, 0].reshape(NB, LCH, 128).transpose(0, 2, 1)
    ).astype(np.float32)
    return {
        "hst": hst,
        "wq": wprep_d(wq),
        "wk": wprep_d(wk),
        "wv": wprep(wv),
        "bq": bq_t,
        "bk": bk_t,
        "bvb": bvb,
        "mask": msk,
    }


def kernel(
    hidden_states,
    attention_mask,
    query_weight,
    query_bias,
    key_weight,
    key_bias,
    value_weight,
    value_bias,
    _trace=False,
):
    hs = np.asarray(hidden_states, dtype=np.float32)
    mask = np.asarray(attention_mask, dtype=np.float32)
    wq = np.asarray(query_weight, dtype=np.float32)
    wk = np.asarray(key_weight, dtype=np.float32)
    wv = np.asarray(value_weight, dtype=np.float32)
    bq = np.asarray(query_bias, dtype=np.float32)
    bk = np.asarray(key_bias, dtype=np.float32)
    bv = np.asarray(value_bias, dtype=np.float32)

    nc = _get_compiled()
    in_maps = []
    for c in range(N_CORES):
        g, b0 = c // 2, NB * (c % 2)
        in_maps.append(_prep_core(hs, mask, wq, wk, wv, bq, bk, bv, g, b0))

    global _COMPILED
    res = None
    for attempt in range(3):
        try:
            res = bass_utils.run_bass_kernel_spmd(
                nc, in_maps, core_ids=list(range(N_CORES)), trace=_trace
            )
            # force materialization so device faults surface here
            for m in res.results:
                for v in m.values():
                    np.asarray(v)
            break
        except Exception:
            if attempt == 2:
                raise
            _COMPILED = None
            nc = _get_compiled()

    out = np.empty((G, B, L, C), dtype=np.float32)
    for c in range(N_CORES):
        g, b0 = c // 2, NB * (c % 2)
        o = res.results[c]["out"]  # [NB, LCH, 128, C] bf16
        out[g, b0 : b0 + NB] = o.reshape(NB, L, C).astype(np.float32)
    if _trace:
        kernel.last_exec_time_ns = res.exec_time_ns
    return out


# revision 32
# speedup vs baseline: 1.0105x; 1.0105x over previous
"""Grouped BERT self-attention on 8 TRN2 NeuronCores.

Problem: G=4 groups, B=4 batch, L=512 seq, C=768 (12 heads x 64).
Sharding: the 16 (g, b) attention problems are embarrassingly parallel;
each core handles one group g = core//2 and two batches. Weights are
per-group so each core loads exactly one group's weights. No collectives.

Per-(g,b) on-chip dataflow (bf16 matmul inputs, fp32 accumulation):
  qT[d,l] = Wq[c,d].T @ hsT[c,l]    (weights in natural layout = lhsT;
                                     bias folded into PSUM->SBUF copy)
  kT[d,l] = Wk[c,d].T @ hsT[c,l]
  v[m,d]  = hsT[c,m].T @ Wv[c,d]    (+bias, stored [m, head, 65] with a
                                     ones column per head -> softmax denom)
  ST[m,l] = kT[d,m].T @ qT[d,l]     (heads paired on partitions 0:64/64:128
                                     -> concurrent PE row-tiles, shared
                                     2-bank PSUM tile)
  E[m,l]  = exp(0.125*ST + mask[m]) (one ACT op per head-pair, bf16 out)
  ctx[l, 2, d+1] = E[m,l].T @ v_aug[m, d+1]  (ctx-direct: E chunk is the
                                     stationary operand, head pair shares
                                     one PSUM bank; column d=64 catches
                                     the softmax denominator)
  out[l,:] = ctx * recip(denom)     (one [128,2] reciprocal + one
                                     broadcast-multiply per pair/l-chunk)

PE emission interleaves score-pair matmuls of unit N with the PV matmuls
of unit N-2 so the in-order PE queue never waits on the ScalarEngine's
exp. Input DMAs are split/staged so the tensors gating the first matmul
group get full HBM bandwidth, and dummy warm-up matmuls hold the PE
activity monitor at full clock until real work arrives.
"""

import numpy as np
import ml_dtypes

import concourse.bacc as bacc
import concourse.bass as bass
import concourse.tile as tile
import concourse.mybir as mybir
from concourse import bass_utils

# avoid FishPath artifact upload in the axon trace path
bass_utils.upload_artifacts = lambda tmpdir: tmpdir

G, B, L, C = 4, 4, 512, 768
NH, DH = 12, 64
NB = 2          # batches per core
CCH = C // 128  # 6 contraction chunks
LCH = L // 128  # 4 seq chunks
N_CORES = 8

BF16 = mybir.dt.bfloat16
F32 = mybir.dt.float32
NPBF16 = ml_dtypes.bfloat16

_COMPILED = None


def _build():
    nc = bacc.Bacc("TRN2", target_bir_lowering=False, debug=False)
    AF = mybir.ActivationFunctionType

    hst_d = nc.declare_dram_parameter("hst", [NB, 128, CCH, L], BF16, isOutput=False)
    wq_d = nc.declare_dram_parameter("wq", [CCH, 128, CCH, 128], BF16, isOutput=False)
    wk_d = nc.declare_dram_parameter("wk", [CCH, 128, CCH, 128], BF16, isOutput=False)
    wv_d = nc.declare_dram_parameter("wv", [128, CCH, C], BF16, isOutput=False)
    bq_d = nc.declare_dram_parameter("bq", [128, CCH], F32, isOutput=False)
    bk_d = nc.declare_dram_parameter("bk", [128, CCH], F32, isOutput=False)
    bvb_d = nc.declare_dram_parameter("bvb", [128, C], BF16, isOutput=False)
    mask_d = nc.declare_dram_parameter("mask", [NB, 128, LCH], F32, isOutput=False)
    out_d = nc.declare_dram_parameter("out", [NB, LCH, 128, C], BF16, isOutput=True)

    with tile.TileContext(nc) as tc:
        with (
            tc.tile_pool(name="wpool", bufs=1) as wpool,
            tc.tile_pool(name="hpool", bufs=2) as hpool,
            tc.tile_pool(name="qkpool", bufs=2) as qkpool,
            tc.tile_pool(name="vpool", bufs=2 * LCH) as vpool,
            tc.tile_pool(name="epool", bufs=12) as epool,
            tc.tile_pool(name="cpool", bufs=2 * LCH) as cpool,
            tc.tile_pool(name="rpool", bufs=8) as rpool,
            tc.tile_pool(name="pqk", bufs=2, space=bass.MemorySpace.PSUM) as pqk,
            tc.tile_pool(name="pss", bufs=2, space=bass.MemorySpace.PSUM) as pss_pool,
            tc.tile_pool(name="ppv", bufs=2, space=bass.MemorySpace.PSUM) as ppv,
        ):
            # ---- persistent constants ----
            wq = wpool.tile([128, CCH, CCH, 128], BF16, tag="wq")
            wk = wpool.tile([128, CCH, CCH, 128], BF16, tag="wk")
            wv = wpool.tile([128, CCH, C], BF16, tag="wv")
            bq = wpool.tile([128, CCH], F32, tag="bq")
            bk = wpool.tile([128, CCH], F32, tag="bk")
            bvb = wpool.tile([128, C], BF16, tag="bvb")
            # one big DMA per tensor (DMA *issue* costs ~0.7us of queue
            # time each); weights on the sync queue, activations on the
            # scalar queue so the first matmul's inputs land in parallel.
            hst, msk, qt, kt, vt, e_of = {}, {}, {}, {}, {}, {}
            for b in range(NB):
                hst[b] = hpool.tile([128, CCH, L], BF16, tag="hst", name=f"hst{b}")
                msk[b] = hpool.tile([128, LCH], F32, tag="mask", name=f"msk{b}")

            # staged input DMAs: wq + hst0 gate the first matmul group, so
            # they transfer first at full HBM bandwidth; wk, wv, hst1 are
            # chained behind earlier transfers via explicit dependency
            # edges. Small tensors ride the otherwise-idle GPSIMD queue.
            h = CCH // 2
            d_wq0 = nc.sync.dma_start(wq[:, 0], wq_d[0])
            d_wk0 = nc.sync.dma_start(wk[:, 0], wk_d[0])
            d_h0 = [nc.scalar.dma_start(hst[0][:, 2 * i : 2 * i + 2], hst_d[0, :, 2 * i : 2 * i + 2])
                    for i in range(3)]
            nc.gpsimd.dma_start(msk[0][:], mask_d[0])
            nc.gpsimd.dma_start(bq[:], bq_d[:])
            nc.gpsimd.dma_start(bk[:], bk_d[:])
            nc.gpsimd.dma_start(bvb[:], bvb_d[:])
            nc.gpsimd.dma_start(msk[1][:], mask_d[1])
            d_wqA = nc.sync.dma_start(wq[:, 1:3], wq_d[1:3].rearrange("j p k d -> p j k d"))
            d_wkA = nc.sync.dma_start(wk[:, 1:3], wk_d[1:3].rearrange("j p k d -> p j k d"))
            d_wqB = nc.sync.dma_start(wq[:, 3:6], wq_d[3:6].rearrange("j p k d -> p j k d"))
            d_wkB = nc.sync.dma_start(wk[:, 3:6], wk_d[3:6].rearrange("j p k d -> p j k d"))
            d_wv = [nc.sync.dma_start(wv[:, 2 * i : 2 * i + 2], wv_d[:, 2 * i : 2 * i + 2])
                    for i in range(3)]
            d_h1 = [nc.gpsimd.dma_start(hst[1][:, 0:h], hst_d[1, :, 0:h]),
                    nc.gpsimd.dma_start(hst[1][:, h:], hst_d[1, :, h:])]
            for i in range(3):
                bass._add_dep_helper(d_wv[i].ins, d_h0[i].ins, True, "wv after hst0")
            for i in range(2):
                bass._add_dep_helper(d_h1[i].ins, d_h0[i].ins, True, "hst1 after hst0")

            def emit_v(b):
                vt[b] = [
                    vpool.tile([128, NH, DH + 1], BF16, tag="v", name=f"v{b}_{t}")
                    for t in range(LCH)
                ]
                for t in range(LCH):
                    for half in range(2):
                        ncol = C // 2  # 384
                        ps = pqk.tile([128, ncol], F32, tag="big", name="psv")
                        for k in range(CCH):
                            nc.tensor.matmul(
                                ps[:],
                                hst[b][:, k, 128 * t : 128 * (t + 1)],
                                wv[:, k, half * ncol : (half + 1) * ncol],
                                start=(k == 0),
                                stop=(k == CCH - 1),
                            )
                        nh2 = NH // 2
                        nc.vector.tensor_add(
                            vt[b][t][:, half * nh2 : (half + 1) * nh2, 0:DH],
                            ps[:].rearrange("p (h d) -> p h d", d=DH),
                            bvb[:, half * ncol : (half + 1) * ncol].rearrange(
                                "p (h d) -> p h d", d=DH
                            ),
                        )
                    nc.vector.memset(vt[b][t][:, :, DH : DH + 1], 1.0)

            def emit_qk_chunk(b, j):
                if j == 0:
                    qt[b] = qkpool.tile([128, CCH, L], BF16, tag="qt", name=f"qt{b}")
                    kt[b] = qkpool.tile([128, CCH, L], BF16, tag="kt", name=f"kt{b}")
                for w, bias, dst in ((wq, bq, qt[b]), (wk, bk, kt[b])):
                    ps = pqk.tile([128, L], F32, tag="big", name="psqk")
                    for k in range(CCH):
                        nc.tensor.matmul(
                            ps[:],
                            w[:, j, k, :],
                            hst[b][:, k, :],
                            start=(k == 0),
                            stop=(k == CCH - 1),
                        )
                    nc.vector.tensor_scalar_add(dst[:, j, :], ps[:], bias[:, j : j + 1])

            def emit_scores_mc(b, hp, mc):
                if mc == 0:
                    e_of[(b, hp)] = [
                        epool.tile([128, 2, L], BF16, tag="e", name=f"e{b}_{hp}_{m}")
                        for m in range(LCH)
                    ]
                e = e_of[(b, hp)]
                ps = pss_pool.tile([128, 2, L], F32, tag="pss", name="pss")
                for h2 in range(2):
                    pr = slice(64 * h2, 64 * (h2 + 1))
                    nc.tensor.matmul(
                        ps[:, h2, :],
                        kt[b][pr, hp, 128 * mc : 128 * (mc + 1)],
                        qt[b][pr, hp, :],
                    )
                nc.scalar.activation(
                    e[mc][:], ps[:], AF.Exp,
                    bias=msk[b][:, mc : mc + 1], scale=0.125,
                )

            def emit_pv_pair(b, hp, lcs):
                # ctx-direct PV for a head pair over l-chunks lcs: one PSUM
                # bank holds both heads' [128, 65] results; denominators at
                # column 64 of each -> one [128,2] reciprocal + one
                # broadcast-multiply into the assembled ctx tile.
                e = e_of[(b, hp)]
                for lc in lcs:
                    pc = ppv.tile([128, 2, DH + 1], F32, tag="pv", name="pc")
                    for h2 in range(2):
                        head = 2 * hp + h2
                        for mc in range(LCH):
                            nc.tensor.matmul(
                                pc[:, h2, :],
                                e[mc][:, h2, 128 * lc : 128 * (lc + 1)],
                                vt[b][mc][:, head, :],
                                start=(mc == 0),
                                stop=(mc == LCH - 1),
                            )
                    rec = rpool.tile([128, 2, 1], F32, tag="rec", name="rec")
                    nc.vector.reciprocal(rec[:], pc[:, :, DH : DH + 1])
                    nc.vector.tensor_mul(
                        ctxs[(b, lc)][:, 2 * hp : 2 * hp + 2, :],
                        pc[:, :, 0:DH],
                        rec[:].broadcast_to((128, 2, DH)),
                    )

            # ---- HAM warm-up: dummy matmuls on garbage SBUF keep the PE
            # active (half-clock) from program start so the activity monitor
            # releases the clock gate before the first real matmul ----
            warm = wpool.tile([128, L], BF16, tag="warm")
            nc.vector.memset(warm[:], 0.0)
            for i in range(24):
                pw = ppv.tile([1, L], F32, tag="pv", name=f"pw{i}")
                nc.tensor.matmul(pw[:], warm[:, 0:1], warm[:])

            # ---- emission schedule ----
            units = []
            for b in range(NB):
                for hp in range(CCH):
                    units.append((b, hp))

            def emit_out(b):
                for lc in range(LCH):
                    flat = ctxs[(b, lc)][:].rearrange("p h d -> p (h d)")
                    nc.sync.dma_start(out_d[b, lc, :, 0 : C // 2], flat[:, 0 : C // 2])
                    nc.gpsimd.dma_start(out_d[b, lc, :, C // 2 :], flat[:, C // 2 :])

            ctxs = {}
            for b in range(NB):
                for lc in range(LCH):
                    ctxs[(b, lc)] = cpool.tile(
                        [128, NH, DH], BF16, tag="ctx", name=f"ctx{b}_{lc}"
                    )
            from collections import deque
            # hand-scheduled prologue: no PV work exists yet to cover the
            # ScalarEngine's exp latency, so interleave qk(0,1) and v(0)
            # between score groups as PE filler
            emit_qk_chunk(0, 0)
            emit_scores_mc(0, 0, 0)
            emit_scores_mc(0, 0, 1)
            emit_qk_chunk(0, 1)
            emit_scores_mc(0, 0, 2)
            emit_scores_mc(0, 0, 3)
            emit_scores_mc(0, 1, 0)
            emit_scores_mc(0, 1, 1)
            emit_v(0)
            emit_scores_mc(0, 1, 2)
            emit_scores_mc(0, 1, 3)
            lag = deque([(0, 0), (0, 1)])
            for b, hp in units[2:]:
                if b == 1 and hp == 0:
                    emit_v(1)
                emit_qk_chunk(b, hp)
                emit_scores_mc(b, hp, 0)
                emit_scores_mc(b, hp, 1)
                if len(lag) >= 2:
                    emit_pv_pair(*lag[0], (0, 1))
                emit_scores_mc(b, hp, 2)
                emit_scores_mc(b, hp, 3)
                if len(lag) >= 2:
                    pp = lag.popleft()
                    emit_pv_pair(*pp, (2, 3))
                    e_of.pop(pp)
                    if pp[1] == CCH - 1:
                        emit_out(pp[0])
                lag.append((b, hp))
            while lag:
                pp = lag.popleft()
                emit_pv_pair(*pp, (0, 1))
                emit_pv_pair(*pp, (2, 3))
                e_of.pop(pp)
                if pp[1] == CCH - 1:
                    emit_out(pp[0])

    nc.compile()
    return nc


def _get_compiled():
    global _COMPILED
    if _COMPILED is None:
        _COMPILED = _build()
    return _COMPILED


def _prep_core(hs, mask, wq, wk, wv, bq, bk, bv, g, b0):
    hs_gb = np.ascontiguousarray(hs[g, b0 : b0 + NB])  # [2, L, C]
    # hst[b, p, j, l] = hs[g, b0+b, l, 128j+p]
    hst = np.ascontiguousarray(
        hs_gb.transpose(0, 2, 1).reshape(NB, CCH, 128, L).transpose(0, 2, 1, 3)
    ).astype(NPBF16)

    def wprep(w):
        # k-chunk-major: [p, k, d] = W[128k+p, d]
        return np.ascontiguousarray(
            w[g].reshape(CCH, 128, C).transpose(1, 0, 2)
        ).astype(NPBF16)

    def wprep_d(w):
        # d-chunk-major: [j, p, k, d'] = W[128k+p, 128j+d']
        return np.ascontiguousarray(
            w[g].reshape(CCH, 128, CCH, 128).transpose(2, 1, 0, 3)
        ).astype(NPBF16)

    bq_t = np.ascontiguousarray(bq[g, 0].reshape(CCH, 128).T).astype(np.float32)
    bk_t = np.ascontiguousarray(bk[g, 0].reshape(CCH, 128).T).astype(np.float32)
    bvb = np.ascontiguousarray(np.broadcast_to(bv[g, 0], (128, C))).astype(NPBF16)
    # mask[b, p, mc] = mask[g, b0+b, 0, 0, 128mc+p]
    msk = np.ascontiguousarray(
        mask[g, b0 : b0 + NB, 0, 0].reshape(NB, LCH, 128).transpose(0, 2, 1)
    ).astype(np.float32)
    return {
        "hst": hst,
        "wq": wprep_d(wq),
        "wk": wprep_d(wk),
        "wv": wprep(wv),
        "bq": bq_t,
        "bk": bk_t,
        "bvb": bvb,
        "mask": msk,
    }


def kernel(
    hidden_states,
    attention_mask,
    query_weight,
    query_bias,
    key_weight,
    key_bias,
    value_weight,
    value_bias,
    _trace=False,
):
    hs = np.asarray(hidden_states, dtype=np.float32)
    mask = np.asarray(attention_mask, dtype=np.float32)
    wq = np.asarray(query_weight, dtype=np.float32)
    wk = np.asarray(key_weight, dtype=np.float32)
    wv = np.asarray(value_weight, dtype=np.float32)
    bq = np.asarray(query_bias, dtype=np.float32)
    bk = np.asarray(key_bias, dtype=np.float32)
    bv = np.asarray(value_bias, dtype=np.float32)

    nc = _get_compiled()
    in_maps = []
    for c in range(N_CORES):
        g, b0 = c // 2, NB * (c % 2)
        in_maps.append(_prep_core(hs, mask, wq, wk, wv, bq, bk, bv, g, b0))

    global _COMPILED
    res = None
    for attempt in range(3):
        try:
            res = bass_utils.run_bass_kernel_spmd(
                nc, in_maps, core_ids=list(range(N_CORES)), trace=_trace
            )
            # force materialization so device faults surface here
            for m in res.results:
                for v in m.values():
                    np.asarray(v)
            break
        except Exception:
            if attempt == 2:
                raise
            _COMPILED = None
            nc = _get_compiled()

    out = np.empty((G, B, L, C), dtype=np.float32)
    for c in range(N_CORES):
        g, b0 = c // 2, NB * (c % 2)
        o = res.results[c]["out"]  # [NB, LCH, 128, C] bf16
        out[g, b0 : b0 + NB] = o.reshape(NB, L, C).astype(np.float32)
    if _trace:
        kernel.last_exec_time_ns = res.exec_time_ns
    return out
